# revision 1
# baseline (speedup 1.0000x reference)
"""AutoCorrelation (Autoformer-style) sparse attention kernel for 8 trn2 cores.

Math (exact refactoring of the reference):
  mean_corr[b,j] = <(sum_i queries[b,i]) @ wq @ wk.T, keys[b,j]> / (H*L)
  top7 delays d_k + softmax weights w_k over mean_corr
  out[b,l]      = sum_k w_k * (values[b] @ wv @ wo)[(l+d_k) % L]
                = (sum_k w_k * values[b,(l+d_k)%L]) @ (wv@wo)

Sharding: core c handles batch b=c//2, output half h=c%2 (rows [h*1024, h*1024+1024)).
Host does layout-only prep (slicing / transpose); all FLOPs run on device.
The two big matmuls (W2 = wv@wo and vmix@W2) run in f32r; their inputs are
produced as real f32r tiles (walrus requires rounded producers).

Hardware constraints honored: a DMA carries at most one sync wait, so every DMA
writes a fresh SBUF region; scratch aliases spent regions of resident packs.
"""

import numpy as np
from contextlib import ExitStack

import concourse.bass as bass
import concourse.bacc as bacc
import concourse.mybir as mybir
import concourse.tile as tile
from concourse import masks
from concourse.bass_utils import run_bass_kernel_spmd

B, L, D, H = 4, 2048, 512, 8
HALF = L // 2          # 1024 output rows per core
KTOP = 7               # max(1, int(log(2048))) = 7
EXT = L + HALF         # values extended along L for wrap-free dynamic slicing
P = 128
FT = D // P            # 4 feature tiles
NT = L // P            # 16 sequence tiles
NH = HALF // P         # 8 output row chunks
F32 = mybir.dt.float32
F32R = mybir.dt.float32r
U32 = mybir.dt.uint32
I32 = mybir.dt.int32
AF = mybir.ActivationFunctionType
ALU = mybir.AluOpType
ENG = mybir.EngineType

# engine split points (DVE vs gpsimd)
SC_DVE = 11            # keys tiles 0..10 scored on DVE, 11..15 on gpsimd
MIX_DVE = 768          # mix columns [0,640) on DVE, [640,1024) on gpsimd


def _build():
    nc = bacc.Bacc()
    q_d = nc.dram_tensor("q", [L, D], F32, kind="ExternalInput")
    k_d = nc.dram_tensor("k", [L, D], F32, kind="ExternalInput")
    vt_d = nc.dram_tensor("vt", [P, FT, L], F32, kind="ExternalInput")
    wq_d = nc.dram_tensor("wq", [D, D], F32, kind="ExternalInput")
    wkT_d = nc.dram_tensor("wkT", [D, D], F32, kind="ExternalInput")
    wvT_d = nc.dram_tensor("wvT", [D, D], F32, kind="ExternalInput")
    wo_d = nc.dram_tensor("wo", [D, D], F32, kind="ExternalInput")
    ident_d = nc.dram_tensor("ident", [P, P], F32, kind="ExternalInput")
    cstc_d = nc.dram_tensor("cstc", [P, 1], F32, kind="ExternalInput")
    cstr_d = nc.dram_tensor("cstr", [1, 2 * P], F32, kind="ExternalInput")
    out_d = nc.dram_tensor("out", [HALF, D], F32, kind="ExternalOutput")

    with tile.TileContext(nc) as tc, ExitStack() as ctx:
        big = ctx.enter_context(tc.tile_pool(name="big", bufs=1))
        sm = ctx.enter_context(tc.tile_pool(name="sm", bufs=1))
        scr = ctx.enter_context(tc.tile_pool(name="scr", bufs=1))
        psA = ctx.enter_context(
            tc.tile_pool(name="psA", bufs=2, space=bass.MemorySpace.PSUM)
        )
        psB = ctx.enter_context(
            tc.tile_pool(name="psB", bufs=2, space=bass.MemorySpace.PSUM)
        )

        qdr = q_d.rearrange("(t p) c -> p t c", p=P)
        kdr = k_d.rearrange("(t p) c -> p t c", p=P)

        # ---- resident input packs; DMAs in priority order, fresh targets -
        wktp = big.tile([P, FT, D], F32, tag="wktp")
        nc.sync.dma_start(wktp[:], wkT_d.rearrange("(m p) c -> p m c", p=P))
        wqp = big.tile([P, FT, D], F32, tag="wqp")
        nc.sync.dma_start(wqp[:], wq_d.rearrange("(m p) c -> p m c", p=P))

        qpack = big.tile([P, NT, D], F32, tag="qpack")
        for j in range(8):
            nc.sync.dma_start(
                qpack[:, 2 * j : 2 * j + 2, :], qdr[:, 2 * j : 2 * j + 2, :]
            )
        kpack = big.tile([P, NT, D], F32, tag="kpack")
        for j in range(8):
            nc.sync.dma_start(
                kpack[:, 2 * j : 2 * j + 2, :], kdr[:, 2 * j : 2 * j + 2, :]
            )

        # values (transposed); circular extension built on ACT
        vt_sb = big.tile([P, FT, EXT], F32, tag="vt")
        nc.sync.dma_start(vt_sb[:, 0:2, 0:L], vt_d[:, 0:2, :])
        nc.sync.dma_start(vt_sb[:, 2:4, 0:L], vt_d[:, 2:4, :])
        nc.scalar.copy(vt_sb[:, :, L:EXT], vt_sb[:, :, 0:HALF])

        wvp = big.tile([P, FT, D], F32, tag="wvp")
        nc.sync.dma_start(wvp[:], wvT_d.rearrange("(m p) c -> p m c", p=P))
        wop = big.tile([P, FT, D], F32, tag="wop")
        nc.sync.dma_start(wop[:], wo_d.rearrange("(m p) c -> p m c", p=P))

        # ---- small packed tiles -----------------------------------------
        aux = sm.tile([P, 352], F32, tag="aux")
        ident = aux[:, 0:P]
        s_tile = aux[:, P : P + NT]
        qsumT = aux[:, 144:148]
        t1T = aux[:, 148:152]
        wbc = aux[:, 152:159]
        ones_col = aux[:, 159:160]
        sT_sb = aux[0:16, 160:288]
        vals8 = aux[0:1, 288:296]
        ex = aux[0:1, 296:303]
        negm = aux[0:1, 303:304]
        se = aux[0:1, 304:305]
        rse = aux[0:1, 305:306]
        w_sb = aux[0:1, 306:313]
        idx8 = aux[0:1, 320:328].bitcast(U32)
        nc.sync.dma_start(ident, ident_d[:])
        nc.sync.dma_start(ones_col, cstc_d[:])
        vec = sm.tile([1, 768], F32, tag="vec")
        qsum_sb = vec[0:1, 0:D]
        ones_row = vec[0:1, 512:640]
        scl_row = vec[0:1, 640:768]
        nc.sync.dma_start(vec[0:1, 512:768], cstr_d[:])

        s_flat = sm.tile([1, L], F32, tag="s_flat")
        u_sb = s_flat[0:1, 0:D]

        # ---- qsum: tree-accumulate q tiles on DVE+gpsimd, then one matmul
        # DVE folds tiles 1..7 into region 0; gpsimd folds 9..15 into 8.
        for t in range(1, 8):
            nc.vector.tensor_tensor(
                qpack[:, 0, :], qpack[:, t, :], qpack[:, 0, :], ALU.add
            )
        for t in range(9, NT):
            nc.vector.tensor_tensor(
                qpack[:, 8, :], qpack[:, t, :], qpack[:, 8, :], ALU.add
            )
        nc.vector.tensor_tensor(
            qpack[:, 0, :], qpack[:, 8, :], qpack[:, 0, :], ALU.add
        )
        ps_qsum = psA.tile([1, D], F32, tag="psa")
        nc.tensor.matmul(ps_qsum[:], ones_col, qpack[:, 0, :], start=True, stop=True)
        nc.scalar.copy(qsum_sb, ps_qsum[:])

        # qsumT [128,4] via 4 tiny K=1 matmuls: out = qsum_chunk^T @ [1]
        for c in range(FT):
            ps_qT = psA.tile([P, 1], F32, tag="psa")
            nc.tensor.matmul(
                ps_qT[:],
                vec[0:1, c * P : (c + 1) * P],
                ones_row[0:1, 0:1],
                start=True,
                stop=True,
            )
            nc.scalar.copy(qsumT[:, c : c + 1], ps_qT[:])

        # ---- t1T = (qsum @ wq) transposed, as [128,4] -------------------
        for jc in range(FT):
            ps_t1 = psA.tile([P, 1], F32, tag="psa")
            for mc in range(FT):
                nc.tensor.matmul(
                    ps_t1[:],
                    wqp[:, mc, jc * P : (jc + 1) * P],
                    qsumT[:, mc : mc + 1],
                    start=(mc == 0),
                    stop=(mc == FT - 1),
                )
            nc.scalar.copy(t1T[:, jc : jc + 1], ps_t1[:])

        # ---- u[1,512] = t1 @ wk.T (fp32) --------------------------------
        ps_u = psA.tile([1, D], F32, tag="psa")
        for mc in range(FT):
            nc.tensor.matmul(
                ps_u[:],
                t1T[:, mc : mc + 1],
                wktp[:, mc, :],
                start=(mc == 0),
                stop=(mc == FT - 1),
            )
        nc.scalar.copy(u_sb, ps_u[:])

        # broadcast u/(H*L) along partitions -> [128,512]
        ps_ub = psA.tile([P, D], F32, tag="psa")
        nc.tensor.matmul(ps_ub[:], scl_row, u_sb, start=True, stop=True)
        ub_sb = sm.tile([P, D], F32, tag="ub")
        nc.scalar.copy(ub_sb[:], ps_ub[:])

        # ---- scores s[128,16]: s[p,t] = <keys[t*128+p], u>/(H*L) --------
        # dead outputs land in spent qpack regions
        for t in range(NT):
            nc.vector.tensor_tensor(
                qpack[:, 1, :], kpack[:, t, :], ub_sb[:], ALU.mult
            )
            nc.vector.tensor_reduce(
                s_tile[:, t : t + 1], qpack[:, 1, :], mybir.AxisListType.X, ALU.add
            )

        # ---- flatten scores to [1,2048]: l = t*128+p --------------------
        ps_sT = psA.tile([NT, P], F32, tag="psa")
        nc.tensor.transpose(ps_sT[:], s_tile, ident)
        nc.scalar.copy(sT_sb[:], ps_sT[:])
        nc.sync.dma_start(s_flat[0:1, :], sT_sb[:])

        # ---- top-8 values + indices (descending), softmax over first 7 --
        nc.vector.max(vals8, s_flat[:])
        nc.vector.max_index(idx8, vals8, s_flat[:])

        nc.vector.tensor_scalar_mul(negm, vals8[0:1, 0:1], -1.0)
        nc.scalar.activation(ex, vals8[0:1, 0:KTOP], AF.Exp, bias=negm)
        nc.vector.tensor_reduce(se, ex, mybir.AxisListType.X, ALU.add)
        nc.vector.reciprocal(rse, se)
        nc.vector.tensor_scalar_mul(w_sb, ex, rse)

        # broadcast weights along partitions -> [128,7]
        ps_wbc = psA.tile([P, KTOP], F32, tag="psa")
        nc.tensor.matmul(ps_wbc[:], ones_row, w_sb, start=True, stop=True)
        nc.scalar.copy(wbc, ps_wbc[:])

        # ---- delays into registers (one batched load per engine) -------
        _, dks = nc.values_load_multi_w_load_instructions(
            idx8[0:1, 0:KTOP].bitcast(I32),
            engines=(ENG.DVE, ENG.Activation),
            min_val=0,
            max_val=L - 1,
            skip_runtime_bounds_check=True,
        )

        # ---- weighted circular mix ---------------------------------------
        # cols [0,MIX_DVE): ACT k0 scaled copy, then DVE stt accumulation
        # cols [MIX_DVE,HALF): pool mul+add pairs in f32 scratch, DVE rounds
        MG = HALF - MIX_DVE
        va = big.tile([P, FT, MIX_DVE], F32, tag="va")
        vb = big.tile([P, FT, MG], F32, tag="vb")
        accg = qpack[:, 2:4, :].rearrange("p a b -> p (a b)").rearrange(
            "p (f l) -> p f l", f=FT
        )
        tmpg = qpack[:, 4:6, :].rearrange("p a b -> p (a b)").rearrange(
            "p (f l) -> p f l", f=FT
        )
        nc.scalar.mul(
            va[:], vt_sb[:, :, bass.ds(dks[0], MIX_DVE)], wbc[:, 0:1]
        )
        nc.scalar.mul(
            accg[:], vt_sb[:, :, bass.ds(dks[0] + MIX_DVE, MG)], wbc[:, 0:1]
        )
        for kk in range(1, KTOP):
            nc.vector.scalar_tensor_tensor(
                va[:],
                vt_sb[:, :, bass.ds(dks[kk], MIX_DVE)],
                wbc[:, kk : kk + 1],
                va[:],
                ALU.mult,
                ALU.add,
            )
            nc.vector.scalar_tensor_tensor(
                accg[:],
                vt_sb[:, :, bass.ds(dks[kk] + MIX_DVE, MG)],
                wbc[:, kk : kk + 1],
                accg[:],
                ALU.mult,
                ALU.add,
            )
        nc.vector.tensor_copy(vb[:], accg[:])

        # ---- W2 = wv @ wo (f32r, chunkwise real-tile casts) -------------
        w2 = big.tile([P, FT, D], F32, tag="w2")
        ps_w2 = [
            psB.tile([P, D], F32, tag="psb", bufs=4, name=f"ps_w2_{i}")
            for i in range(FT)
        ]
        for mc in range(FT):
            for ic in range(FT):
                nc.tensor.matmul(
                    ps_w2[ic][:],
                    wvp[:, mc, ic * P : (ic + 1) * P],
                    wop[:, mc, :],
                    start=(mc == 0),
                    stop=(mc == FT - 1),
                )
        for ic in range(FT):
            nc.scalar.copy(w2[:, ic, :], ps_w2[ic][:])

        # ---- out rows: out[l,:] = sum_f vmixT[f,l] * W2[f,:] ------------
        # out staging aliases the spent second half of kpack
        for lc in range(NH):
            ps_out = psB.tile([P, D], F32, tag="psb", bufs=4)
            for ft in range(FT):
                src = (
                    va[:, ft, lc * P : (lc + 1) * P]
                    if (lc + 1) * P <= MIX_DVE
                    else vb[:, ft, lc * P - MIX_DVE : (lc + 1) * P - MIX_DVE]
                )
                nc.tensor.matmul(
                    ps_out[:],
                    src,
                    w2[:, ft, :],
                    start=(ft == 0),
                    stop=(ft == FT - 1),
                )
            ot = kpack[:, 8 + lc, :]
            nc.scalar.copy(ot, ps_out[:])
            nc.sync.dma_start(out_d[lc * P : (lc + 1) * P, :], ot)

    return nc


_IDENT = np.eye(P, dtype=np.float32)
_CSTC = np.ones((P, 1), np.float32)
_CSTR = np.concatenate(
    [np.ones((1, P), np.float32), np.full((1, P), 1.0 / (H * L), np.float32)], axis=1
)
_NC = None
TRACE = False
_LAST_RESULTS = None


def _get_nc():
    global _NC
    if _NC is None:
        _NC = _build()
        _NC.finalize()
    return _NC


def kernel(queries, keys, values, wq, wk, wv, wo):
    nc = _get_nc()
    wkT = np.ascontiguousarray(wk.T)
    wvT = np.ascontiguousarray(wv.T)
    in_maps = []
    for c in range(8):
        b, h = divmod(c, 2)
        vrot = np.roll(values[b], -h * HALF, axis=0)
        vte = np.ascontiguousarray(vrot.T.reshape(FT, P, L).transpose(1, 0, 2))
        in_maps.append(
            {
                "q": np.ascontiguousarray(queries[b]),
                "k": np.ascontiguousarray(keys[b]),
                "vt": vte,
                "wq": np.ascontiguousarray(wq),
                "wkT": wkT,
                "wvT": wvT,
                "wo": np.ascontiguousarray(wo),
                "ident": _IDENT,
                "cstc": _CSTC,
                "cstr": _CSTR,
            }
        )
    global _LAST_RESULTS
    res = run_bass_kernel_spmd(nc, in_maps, list(range(8)), trace=TRACE)
    _LAST_RESULTS = res
    out = np.empty((B, L, D), np.float32)
    for c in range(8):
        b, h = divmod(c, 2)
        out[b, h * HALF : (h + 1) * HALF] = res.results[c]["out"]
    return out



# revision 4
# speedup vs baseline: 3.2527x; 3.2527x over previous
"""AutoCorrelation (Autoformer-style) sparse attention kernel for 8 trn2 cores.

Math (exact refactoring of the reference):
  mean_corr[b,j] = <((sum_i q[b,i]) @ wq) @ wk.T, keys[b,j]> / (H*L)
  top7 delays d_k + softmax weights w_k over mean_corr
  out[b,l]      = (sum_k w_k * values[b,(l+d_k)%L]) @ (wv@wo)

Sharding: core c handles batch b=c//2, output half h=c%2.

v2 design (cost-model driven):
  - bf16 on the wire for all inputs (error budget 2e-2; measured ~5e-3)
  - input DMAs split across three queues (SP / Activation / gpsimd-SWDGE),
    constants first
  - score path entirely on PE as bf16 matmuls producing s in [1, L] layout
  - topk in f32 on DVE (Max + MaxIndex)
  - softmax: exp on ACT (no bias arg), 1/sum folded into the final output
    copy scale, raw-exp weights broadcast via PE
  - circular weighted mix split across DVE (stt), Pool (stt), and PE
    (PSUM-accumulated matmuls with w_k-scaled identity lhs)
  - final projection via W2 = wv@wo in bf16 on PE, out chunks copied f32
    with the 1/sum scale and DMA'd per chunk
"""

import numpy as np
from contextlib import ExitStack

import concourse.bass as bass
import concourse.bacc as bacc
import concourse.mybir as mybir
import concourse.tile as tile
from concourse.bass_utils import run_bass_kernel_spmd

B, L, D, H = 4, 2048, 512, 8
HALF = L // 2          # 1024 output rows per core
KTOP = 7               # max(1, int(log(2048))) = 7
EXT = L + HALF         # values extended along L for wrap-free dynamic slicing
P = 128
FT = D // P            # 4 feature tiles
NT = L // P            # 16 sequence tiles
NCH = HALF // P        # 8 output row chunks
F32 = mybir.dt.float32
BF16 = mybir.dt.bfloat16
U32 = mybir.dt.uint32
I32 = mybir.dt.int32
AF = mybir.ActivationFunctionType
ALU = mybir.AluOpType
ENG = mybir.EngineType

# mix chunk assignment: engine per 128-row output chunk
MIX_ENG = ["dve", "dve", "pool", "pool", "pool", "pe", "pe", "pe"]


def _build():
    nc = bacc.Bacc()
    q_d = nc.dram_tensor("q", [L, D], BF16, kind="ExternalInput")
    kt_d = nc.dram_tensor("kt", [D, L], BF16, kind="ExternalInput")
    vt_d = nc.dram_tensor("vt", [D, L], BF16, kind="ExternalInput")
    wq_d = nc.dram_tensor("wq", [D, D], BF16, kind="ExternalInput")
    wkT_d = nc.dram_tensor("wkT", [D, D], BF16, kind="ExternalInput")
    wvT_d = nc.dram_tensor("wvT", [D, D], BF16, kind="ExternalInput")
    wo_d = nc.dram_tensor("wo", [D, D], BF16, kind="ExternalInput")
    cbf_d = nc.dram_tensor("cbf", [P, 257], BF16, kind="ExternalInput")
    cf32_d = nc.dram_tensor("cf32", [1, P], F32, kind="ExternalInput")
    out_d = nc.dram_tensor("out", [HALF, D], F32, kind="ExternalOutput")

    qdr = q_d.rearrange("(t p) c -> p t c", p=P)
    ktdr = kt_d.rearrange("(f p) l -> p f l", p=P)
    vtdr = vt_d.rearrange("(f p) l -> p f l", p=P)

    with tile.TileContext(nc) as tc, ExitStack() as ctx:
        big = ctx.enter_context(tc.tile_pool(name="big", bufs=1))
        sm = ctx.enter_context(tc.tile_pool(name="sm", bufs=1))
        psp = ctx.enter_context(
            tc.tile_pool(name="psp", bufs=4, space=bass.MemorySpace.PSUM)
        )

        # ---- constants (first on SP so nothing blocks on them) ----------
        cbf = sm.tile([P, 257], BF16, tag="cbf")
        nc.sync.dma_start(cbf[:], cbf_d[:])
        ident = cbf[:, 0:128]
        ones_col = cbf[:, 128:129]
        one_one = cbf[0:1, 128:129]
        ones_row = cbf[0:1, 129:257]
        cf32 = sm.tile([1, P], F32, tag="cf32")
        nc.sync.dma_start(cf32[:], cf32_d[:])
        ones_row_f = cf32[0:1, 0:128]

        # ---- input packs: SP queue --------------------------------------
        qp = big.tile([P, NT, D], BF16, tag="qp")
        for j in range(4):
            nc.sync.dma_start(
                qp[:, 2 * j : 2 * j + 2, :], qdr[:, 2 * j : 2 * j + 2, :]
            )
        wqp = big.tile([P, FT, D], BF16, tag="wqp")
        nc.sync.dma_start(wqp[:], wq_d.rearrange("(m p) c -> p m c", p=P))
        wktp = big.tile([P, FT, D], BF16, tag="wktp")
        nc.sync.dma_start(wktp[:], wkT_d.rearrange("(m p) c -> p m c", p=P))

        # ---- ACT queue: q second half, then kT ---------------------------
        for j in range(4, 8):
            nc.scalar.dma_start(
                qp[:, 2 * j : 2 * j + 2, :], qdr[:, 2 * j : 2 * j + 2, :]
            )
        ktp = big.tile([P, FT, L], BF16, tag="ktp")
        for f in range(FT):
            nc.scalar.dma_start(ktp[:, f : f + 1, :], ktdr[:, f : f + 1, :])

        # ---- gpsimd (SWDGE) queue: values + W2 weights -------------------
        vt_sb = big.tile([P, FT, EXT], BF16, tag="vt")
        nc.gpsimd.dma_start(vt_sb[:, 0:2, 0:L], vtdr[:, 0:2, :])
        nc.gpsimd.dma_start(vt_sb[:, 2:4, 0:L], vtdr[:, 2:4, :])
        wvp = big.tile([P, FT, D], BF16, tag="wvp")
        nc.gpsimd.dma_start(wvp[:], wvT_d.rearrange("(m p) c -> p m c", p=P))
        wop = big.tile([P, FT, D], BF16, tag="wop")
        nc.gpsimd.dma_start(wop[:], wo_d.rearrange("(m p) c -> p m c", p=P))
        # circular extension (gpsimd, after its DMAs)
        nc.gpsimd.tensor_copy(vt_sb[:, :, L:EXT], vt_sb[:, :, 0:HALF])

        # ---- small tiles -------------------------------------------------
        aux2 = sm.tile([P, 48], BF16, tag="aux2")
        qsT = aux2[:, 0:4]
        t1T = aux2[:, 4:8]
        uT = aux2[:, 8:12]
        exbf = aux2[0:1, 20:28]
        wbc = sm.tile([P, KTOP], F32, tag="wbc")
        rse_bc = sm.tile([P, 1], F32, tag="rsebc")
        srow = sm.tile([1, 3 * D], BF16, tag="srow")
        qsum_sb = srow[0:1, 0:D]
        t1_sb = srow[0:1, D : 2 * D]
        u_sb = srow[0:1, 2 * D : 3 * D]
        sfl = sm.tile([1, L], F32, tag="sfl")
        aux3 = sm.tile([1, 16], F32, tag="aux3")
        vals8 = aux3[0:1, 0:8]
        exin = aux3[0:1, 8:15]
        se = aux3[0:1, 15:16]
        rse = sm.tile([1, 1], F32, tag="rse")
        idx8 = sm.tile([1, 8], U32, tag="idx8")
        sid = sm.tile([P, KTOP, P], BF16, tag="sid")  # w_k-scaled idents
        mixs = big.tile([P, NCH, FT, P], BF16, tag="mixs")  # mixed chunks (sbuf)
        ostg = sm.tile([P, 2, D], F32, tag="ostg")   # out staging (2 buffers)

        # ---- qsum on PE (chases q DMA arrival) ---------------------------
        ps_qsum = psp.tile([1, D], F32, tag="a")
        for t in range(NT):
            nc.tensor.matmul(
                ps_qsum[:], ones_col, qp[:, t, :], start=(t == 0), stop=(t == NT - 1)
            )
        nc.vector.tensor_copy(qsum_sb, ps_qsum[:])

        # qsumT [128,4] via 4 tiny matmuls (lhs=[1,128] chunk, rhs=[1,1] one)
        ps_qT = psp.tile([P, FT], F32, tag="a")
        for c in range(FT):
            nc.tensor.matmul(
                ps_qT[:, c : c + 1],
                qsum_sb[0:1, c * P : (c + 1) * P],
                one_one,
                start=True,
                stop=True,
                skip_group_check=True,
            )
        nc.vector.tensor_copy(qsT[:], ps_qT[:])

        # t1 = qsum @ wq  (row form)
        ps_t1 = psp.tile([1, D], F32, tag="a")
        for mc in range(FT):
            nc.tensor.matmul(
                ps_t1[:], qsT[:, mc : mc + 1], wqp[:, mc, :],
                start=(mc == 0), stop=(mc == FT - 1),
            )
        nc.vector.tensor_copy(t1_sb, ps_t1[:])
        ps_t1T = psp.tile([P, FT], F32, tag="a")
        for c in range(FT):
            nc.tensor.matmul(
                ps_t1T[:, c : c + 1],
                t1_sb[0:1, c * P : (c + 1) * P],
                one_one,
                start=True,
                stop=True,
                skip_group_check=True,
            )
        nc.vector.tensor_copy(t1T[:], ps_t1T[:])

        # u = t1 @ wk.T  (row form), then uT
        ps_u = psp.tile([1, D], F32, tag="a")
        for mc in range(FT):
            nc.tensor.matmul(
                ps_u[:], t1T[:, mc : mc + 1], wktp[:, mc, :],
                start=(mc == 0), stop=(mc == FT - 1),
            )
        nc.vector.tensor_copy(u_sb, ps_u[:])
        ps_uT = psp.tile([P, FT], F32, tag="a")
        for c in range(FT):
            nc.tensor.matmul(
                ps_uT[:, c : c + 1],
                u_sb[0:1, c * P : (c + 1) * P],
                one_one,
                start=True,
                stop=True,
                skip_group_check=True,
            )
        nc.vector.tensor_copy(uT[:], ps_uT[:])

        # ---- scores s[1,2048] on PE: 4 psum banks, accumulate over f ----
        # note 1/(H*L) folded into uT? no: fold into score copy scale below
        ps_s = [psp.tile([1, D], F32, tag="a", name=f"ps_s{i}") for i in range(FT)]
        for f in range(FT):
            for lc in range(FT):
                nc.tensor.matmul(
                    ps_s[lc][:],
                    uT[:, f : f + 1],
                    ktp[:, f, lc * D : (lc + 1) * D],
                    start=(f == 0),
                    stop=(f == FT - 1),
                )
        # copy scores to flat f32 row, scaling by 1/(H*L)
        # (scale keeps exp() in the same range as the reference)
        for lc in range(FT):
            eng = nc.vector if lc % 2 == 0 else nc.gpsimd
            eng.tensor_scalar_mul(
                sfl[0:1, lc * D : (lc + 1) * D], ps_s[lc][:], 1.0 / (H * L)
            )

        # ---- top-8 + raw-exp weights ------------------------------------
        nc.vector.max(vals8, sfl[:])
        nc.vector.max_index(idx8[:], vals8, sfl[:])

        nc.vector.tensor_scalar_sub(exin, vals8[0:1, 0:KTOP], vals8[0:1, 0:1])
        nc.scalar.activation(exbf[0:1, 0:KTOP], exin, AF.Exp)
        nc.scalar.activation(aux3[0:1, 8:15], exin, AF.Exp)  # f32 copy for sum
        nc.vector.tensor_reduce(se, aux3[0:1, 8:15], mybir.AxisListType.X, ALU.add)
        nc.vector.reciprocal(rse[:], se)

        # broadcasts: wbc [128,7] bf16 (raw exps), rse_bc [128,1] f32
        ps_w = psp.tile([P, 8], F32, tag="b")
        nc.tensor.matmul(ps_w[:, 0:KTOP], ones_row, exbf[0:1, 0:KTOP],
                         start=True, stop=True)
        nc.vector.tensor_copy(wbc[:], ps_w[:, 0:KTOP])
        ps_r = psp.tile([P, 1], F32, tag="b")
        nc.tensor.matmul(ps_r[:], ones_row_f, rse[:], start=True, stop=True)
        nc.vector.tensor_copy(rse_bc[:], ps_r[:])

        # scaled idents for the PE mix (k=0 uses ident directly: exp(0)=1)
        for k in range(1, KTOP):
            nc.vector.tensor_scalar_mul(sid[:, k, :], ident, wbc[:, k : k + 1])

        # ---- delay registers on DVE / Pool / PE -------------------------
        _, dks = nc.values_load_multi_w_load_instructions(
            idx8[0:1, 0:KTOP].bitcast(I32),
            engines=(ENG.DVE, ENG.Pool, ENG.PE),
            min_val=0,
            max_val=L - 1,
            skip_runtime_bounds_check=True,
        )

        # ---- weighted circular mix, chunked; then final matmul ----------
        w2 = big.tile([P, FT, D], BF16, tag="w2")
        ps_w2 = [psp.tile([P, D], F32, tag="b", name=f"ps_w2_{i}") for i in range(FT)]
        for mc in range(FT):
            for ic in range(FT):
                nc.tensor.matmul(
                    ps_w2[ic][:],
                    wvp[:, mc, ic * P : (ic + 1) * P],
                    wop[:, mc, :],
                    start=(mc == 0),
                    stop=(mc == FT - 1),
                )
        for ic in range(FT):
            eng = nc.vector if ic % 2 == 0 else nc.gpsimd
            eng.tensor_copy(w2[:, ic, :], ps_w2[ic][:])

        for ch in range(NCH):
            base = ch * P
            eng = MIX_ENG[ch]
            mslice = mixs[:, ch, :, :]
            if eng == "pe":
                ps_m = psp.tile([P, D], F32, tag="b", name=f"psm{ch}")
                for k in range(KTOP):
                    lhs = ident if k == 0 else sid[:, k, :]
                    nc.tensor.matmul(
                        ps_m[:],
                        lhs,
                        vt_sb[:, :, bass.ds(dks[k] + base, P)],
                        start=(k == 0),
                        stop=(k == KTOP - 1),
                    )
                nc.vector.tensor_copy(mslice, ps_m[:])
            else:
                e = nc.vector if eng == "dve" else nc.gpsimd
                e.tensor_copy(mslice, vt_sb[:, :, bass.ds(dks[0] + base, P)])
                for k in range(1, KTOP):
                    e.scalar_tensor_tensor(
                        mslice,
                        vt_sb[:, :, bass.ds(dks[k] + base, P)],
                        wbc[:, k : k + 1],
                        mslice,
                        ALU.mult,
                        ALU.add,
                    )
            # final: out_chunk = mix_chunk^T @ W2, scaled by rse in the copy
            ps_o = psp.tile([P, D], F32, tag="b", name=f"pso{ch}")
            for ft in range(FT):
                nc.tensor.matmul(
                    ps_o[:],
                    mixs[:, ch, ft, :],
                    w2[:, ft, :],
                    start=(ft == 0),
                    stop=(ft == FT - 1),
                )
            stg = ostg[:, ch % 2, :]
            nc.scalar.mul(stg, ps_o[:], rse_bc[:])
            nc.sync.dma_start(out_d[base : base + P, :], stg)

    return nc


_NC = None
TRACE = False
_LAST_RESULTS = None


def _get_nc():
    global _NC
    if _NC is None:
        _NC = _build()
        _NC.finalize()
    return _NC


def _consts():
    import ml_dtypes

    cbf = np.zeros((P, 257), ml_dtypes.bfloat16)
    cbf[:, 0:128] = np.eye(P, dtype=np.float32)
    cbf[:, 128:257] = 1.0
    cf32 = np.ones((1, P), np.float32)
    return cbf, cf32


def kernel(queries, keys, values, wq, wk, wv, wo):
    import ml_dtypes

    nc = _get_nc()
    bf = ml_dtypes.bfloat16
    wq_b = np.ascontiguousarray(wq, dtype=bf)
    wkT_b = np.ascontiguousarray(wk.T, dtype=bf)
    wvT_b = np.ascontiguousarray(wv.T, dtype=bf)
    wo_b = np.ascontiguousarray(wo, dtype=bf)
    cbf, cf32 = _consts()
    in_maps = []
    for c in range(8):
        b, h = divmod(c, 2)
        vrot = np.roll(values[b], -h * HALF, axis=0)
        in_maps.append(
            {
                "q": np.ascontiguousarray(queries[b], dtype=bf),
                "kt": np.ascontiguousarray(keys[b].T, dtype=bf),
                "vt": np.ascontiguousarray(vrot.T, dtype=bf),
                "wq": wq_b,
                "wkT": wkT_b,
                "wvT": wvT_b,
                "wo": wo_b,
                "cbf": cbf,
                "cf32": cf32,
            }
        )
    global _LAST_RESULTS
    res = run_bass_kernel_spmd(nc, in_maps, list(range(8)), trace=TRACE)
    _LAST_RESULTS = res
    out = np.empty((B, L, D), np.float32)
    for c in range(8):
        b, h = divmod(c, 2)
        out[b, h * HALF : (h + 1) * HALF] = res.results[c]["out"]
    return out


# revision 6
# speedup vs baseline: 4.0967x; 1.2595x over previous
"""AutoCorrelation (Autoformer-style) sparse attention kernel for 8 trn2 cores.

Math (exact refactoring of the reference):
  mean_corr[b,j] = <((sum_i q[b,i]) @ wq) @ wk.T, keys[b,j]> / (H*L)
  top7 delays d_k + softmax weights w_k over mean_corr
  out[b,l]      = (sum_k w_k * values[b,(l+d_k)%L]) @ (wv@wo)

Sharding: core c handles batch b=c//2, output half h=c%2.

v3 schedule (cost-model driven):
  - bf16 on the wire; input DMAs split over SP / ACT / gpsimd queues with
    q tiles interleaved SP/ACT so the PE qsum chain chases arrivals
  - score path on PE: qsum -> qsumT -> t1 -> t1T -> u -> uT -> s[1,2048]
    (tiny transpose hops via [1,128]x[1,1] matmuls, psum copies on DVE)
  - topk f32 on DVE (Max + MaxIndex); softmax prep on Pool/ACT so DVE goes
    straight from MaxIndex into its mix chunks
  - PE kept at full p-state through the topk window with dummy matmuls
  - mix: DVE 2 chunks (stt), Pool 3 chunks (stt, k0 copies on ACT),
    PE 3 chunks (psum-accumulated matmuls, w_k-scaled identity lhs)
  - finals on PE in mix-readiness order; out = psum copy scaled by 1/sum
    of exps (ACT) then DMA per chunk on SP
"""

import numpy as np
from contextlib import ExitStack

import concourse.bass as bass
import concourse.bacc as bacc
import concourse.mybir as mybir
import concourse.tile as tile
from concourse.bass_utils import run_bass_kernel_spmd

B, L, D, H = 4, 2048, 512, 8
HALF = L // 2
KTOP = 7
EXT = L + HALF
P = 128
FT = D // P
NT = L // P
NCH = HALF // P
F32 = mybir.dt.float32
BF16 = mybir.dt.bfloat16
U32 = mybir.dt.uint32
I32 = mybir.dt.int32
AF = mybir.ActivationFunctionType
ALU = mybir.AluOpType
ENG = mybir.EngineType

N_DUMMY = 21  # PE warm-keepers spanning the Max/MaxIndex window


def _build():
    nc = bacc.Bacc()
    q_d = nc.dram_tensor("q", [L, D], BF16, kind="ExternalInput")
    kt_d = nc.dram_tensor("kt", [D, L], BF16, kind="ExternalInput")
    vt_d = nc.dram_tensor("vt", [D, L], BF16, kind="ExternalInput")
    wq_d = nc.dram_tensor("wq", [D, D], BF16, kind="ExternalInput")
    wkT_d = nc.dram_tensor("wkT", [D, D], BF16, kind="ExternalInput")
    wvT_d = nc.dram_tensor("wvT", [D, D], BF16, kind="ExternalInput")
    wo_d = nc.dram_tensor("wo", [D, D], BF16, kind="ExternalInput")
    cbf_d = nc.dram_tensor("cbf", [P, 257], BF16, kind="ExternalInput")
    cf32_d = nc.dram_tensor("cf32", [1, P], F32, kind="ExternalInput")
    out_d = nc.dram_tensor("out", [HALF, D], F32, kind="ExternalOutput")

    qdr = q_d.rearrange("(t p) c -> p t c", p=P)
    ktdr = kt_d.rearrange("(f p) l -> p f l", p=P)
    vtdr = vt_d.rearrange("(f p) l -> p f l", p=P)

    with tile.TileContext(nc) as tc, ExitStack() as ctx:
        big = ctx.enter_context(tc.tile_pool(name="big", bufs=1))
        sm = ctx.enter_context(tc.tile_pool(name="sm", bufs=1))
        psp = ctx.enter_context(
            tc.tile_pool(name="psp", bufs=1, space=bass.MemorySpace.PSUM)
        )

        # ---- DMA plan ----------------------------------------------------
        # SP : consts, q pairs 0/2/4/6, wqp, wktp, kT[3]
        # ACT: q pairs 1/3/5/7, kT[0..2]
        # Pool(SWDGE): vt, wvp, wop
        cbf = sm.tile([P, 257], BF16, tag="cbf")
        nc.sync.dma_start(cbf[:], cbf_d[:])
        ident = cbf[:, 0:128]
        ones_col = cbf[:, 128:129]
        one_one = cbf[0:1, 128:129]
        ones_row = cbf[0:1, 129:257]
        cf32 = sm.tile([1, P], F32, tag="cf32")
        nc.sync.dma_start(cf32[:], cf32_d[:])
        ones_row_f = cf32[0:1, 0:128]

        qp = big.tile([P, NT, D], BF16, tag="qp")
        ktp = big.tile([P, FT, L], BF16, tag="ktp")
        for j in range(4):  # SP: tiles 4j,4j+1; ACT: tiles 4j+2,4j+3
            nc.sync.dma_start(
                qp[:, 4 * j : 4 * j + 2, :], qdr[:, 4 * j : 4 * j + 2, :]
            )
            nc.scalar.dma_start(
                qp[:, 4 * j + 2 : 4 * j + 4, :], qdr[:, 4 * j + 2 : 4 * j + 4, :]
            )
        wqp = big.tile([P, FT, D], BF16, tag="wqp")
        nc.sync.dma_start(wqp[:], wq_d.rearrange("(m p) c -> p m c", p=P))
        wktp = big.tile([P, FT, D], BF16, tag="wktp")
        nc.sync.dma_start(wktp[:], wkT_d.rearrange("(m p) c -> p m c", p=P))
        for f in range(3):
            nc.scalar.dma_start(ktp[:, f : f + 1, :], ktdr[:, f : f + 1, :])
        nc.sync.dma_start(ktp[:, 3:4, :], ktdr[:, 3:4, :])

        vt_sb = big.tile([P, FT, EXT], BF16, tag="vt")
        nc.gpsimd.dma_start(vt_sb[:, 0:2, 0:L], vtdr[:, 0:2, :])
        nc.gpsimd.dma_start(vt_sb[:, 2:4, 0:L], vtdr[:, 2:4, :])
        wvp = big.tile([P, FT, D], BF16, tag="wvp")
        nc.gpsimd.dma_start(wvp[:], wvT_d.rearrange("(m p) c -> p m c", p=P))
        wop = big.tile([P, FT, D], BF16, tag="wop")
        nc.gpsimd.dma_start(wop[:], wo_d.rearrange("(m p) c -> p m c", p=P))
        nc.gpsimd.tensor_copy(vt_sb[:, :, L:EXT], vt_sb[:, :, 0:HALF])

        # ---- small tiles -------------------------------------------------
        aux2 = sm.tile([P, 32], BF16, tag="aux2")
        qsT = aux2[:, 0:4]
        t1T = aux2[:, 4:8]
        uT = aux2[:, 8:12]
        exbf = aux2[0:1, 16:24]
        wbc = sm.tile([P, KTOP], F32, tag="wbc")
        rse_bc = sm.tile([P, 1], F32, tag="rsebc")
        srow = sm.tile([1, 3 * D], BF16, tag="srow")
        qsum_sb = srow[0:1, 0:D]
        t1_sb = srow[0:1, D : 2 * D]
        u_sb = srow[0:1, 2 * D : 3 * D]
        sfl = sm.tile([1, L], F32, tag="sfl")
        aux3 = sm.tile([1, 24], F32, tag="aux3")
        vals8 = aux3[0:1, 0:8]
        exin = aux3[0:1, 8:15]
        se = aux3[0:1, 15:16]
        exf = aux3[0:1, 16:23]
        rse = sm.tile([1, 1], F32, tag="rse")
        idx8 = sm.tile([1, 8], U32, tag="idx8")
        sid = sm.tile([P, KTOP, P], BF16, tag="sid")
        mixs = big.tile([P, NCH, FT, P], BF16, tag="mixs")
        ostg = sm.tile([P, 3, D], F32, tag="ostg")

        # ---- score path on PE -------------------------------------------
        ps_qsum = psp.tile([1, D], F32, tag="a", bufs=2)
        for t in range(NT):
            nc.tensor.matmul(
                ps_qsum[:], ones_col, qp[:, t, :], start=(t == 0), stop=(t == NT - 1)
            )
        nc.vector.tensor_copy(qsum_sb, ps_qsum[:])

        ps_qT = psp.tile([P, FT], F32, tag="a", bufs=2)
        for c in range(FT):
            nc.tensor.matmul(
                ps_qT[:, c : c + 1],
                qsum_sb[0:1, c * P : (c + 1) * P],
                one_one,
                start=True, stop=True, skip_group_check=True,
            )
        nc.vector.tensor_copy(qsT[:], ps_qT[:])

        ps_t1 = psp.tile([1, D], F32, tag="a", bufs=2)
        for mc in range(FT):
            nc.tensor.matmul(
                ps_t1[:], qsT[:, mc : mc + 1], wqp[:, mc, :],
                start=(mc == 0), stop=(mc == FT - 1),
            )
        nc.vector.tensor_copy(t1_sb, ps_t1[:])
        ps_t1T = psp.tile([P, FT], F32, tag="a", bufs=2)
        for c in range(FT):
            nc.tensor.matmul(
                ps_t1T[:, c : c + 1],
                t1_sb[0:1, c * P : (c + 1) * P],
                one_one,
                start=True, stop=True, skip_group_check=True,
            )
        nc.vector.tensor_copy(t1T[:], ps_t1T[:])

        ps_u = psp.tile([1, D], F32, tag="a", bufs=2)
        for mc in range(FT):
            nc.tensor.matmul(
                ps_u[:], t1T[:, mc : mc + 1], wktp[:, mc, :],
                start=(mc == 0), stop=(mc == FT - 1),
            )
        nc.vector.tensor_copy(u_sb, ps_u[:])
        ps_uT = psp.tile([P, FT], F32, tag="a", bufs=2)
        for c in range(FT):
            nc.tensor.matmul(
                ps_uT[:, c : c + 1],
                u_sb[0:1, c * P : (c + 1) * P],
                one_one,
                start=True, stop=True, skip_group_check=True,
            )
        nc.vector.tensor_copy(uT[:], ps_uT[:])

        # scores: 4 psum banks, f-major so each pass chases its kT chunk
        ps_s = [
            psp.tile([1, D], F32, tag="s", bufs=4, name=f"ps_s{i}")
            for i in range(FT)
        ]
        for f in range(FT):
            for lc in range(FT):
                nc.tensor.matmul(
                    ps_s[lc][:],
                    uT[:, f : f + 1],
                    ktp[:, f, lc * D : (lc + 1) * D],
                    start=(f == 0),
                    stop=(f == FT - 1),
                )
        # copies to flat f32 with 1/(H*L) scale: DVE banks 0,2; Pool 1,3
        for lc in range(FT):
            eng = nc.vector if lc % 2 == 0 else nc.gpsimd
            eng.tensor_scalar_mul(
                sfl[0:1, lc * D : (lc + 1) * D], ps_s[lc][:], 1.0 / (H * L)
            )

        # ---- W2 = wv @ wo on PE, ic-outer (2 rotating banks) ------------
        w2 = big.tile([P, FT, D], BF16, tag="w2")
        for ic in range(FT):
            ps_w2 = psp.tile([P, D], F32, tag="b", bufs=2, name=f"ps_w2_{ic}")
            for mc in range(FT):
                nc.tensor.matmul(
                    ps_w2[:],
                    wvp[:, mc, ic * P : (ic + 1) * P],
                    wop[:, mc, :],
                    start=(mc == 0),
                    stop=(mc == FT - 1),
                )
            nc.gpsimd.tensor_copy(w2[:, ic, :], ps_w2[:])

        # ---- topk on DVE; softmax prep on Pool/ACT ----------------------
        nc.vector.max(vals8, sfl[:])
        nc.vector.max_index(idx8[:], vals8, sfl[:])

        nc.gpsimd.tensor_scalar_sub(exin, vals8[0:1, 0:KTOP], vals8[0:1, 0:1])
        nc.scalar.activation(exf, exin, AF.Exp, accum_out=se)
        nc.gpsimd.tensor_copy(exbf[0:1, 0:KTOP], exf)
        nc.gpsimd.tensor_tensor(rse[:], cf32[0:1, 0:1], se, ALU.divide)

        # broadcasts on PE (emitted before dummies)
        ps_w = psp.tile([P, 8], F32, tag="a", bufs=2)
        nc.tensor.matmul(
            ps_w[:, 0:KTOP], ones_row, exbf[0:1, 0:KTOP], start=True, stop=True
        )
        nc.gpsimd.tensor_copy(wbc[:], ps_w[:, 0:KTOP])
        ps_r = psp.tile([P, 1], F32, tag="a", bufs=2)
        nc.tensor.matmul(ps_r[:], ones_row_f, rse[:], start=True, stop=True)
        nc.gpsimd.tensor_copy(rse_bc[:], ps_r[:])

        # scaled idents on Pool (k=0 uses plain ident)
        for k in range(1, KTOP):
            nc.gpsimd.tensor_scalar_mul(sid[:, k, :], ident, wbc[:, k : k + 1])

        # PE warm-keepers: independent matmuls through the topk window
        ps_dum = psp.tile([P, D], F32, tag="b", bufs=2)
        for i in range(N_DUMMY):
            nc.tensor.matmul(
                ps_dum[:], ident, wop[:, 0, :], start=True, stop=True,
                skip_group_check=True,
            )

        # ---- delay registers --------------------------------------------
        _, dks = nc.values_load_multi_w_load_instructions(
            idx8[0:1, 0:KTOP].bitcast(I32),
            engines=(ENG.DVE, ENG.Pool, ENG.PE, ENG.Activation),
            min_val=0,
            max_val=L - 1,
            skip_runtime_bounds_check=True,
        )

        # ---- mix + finals ------------------------------------------------
        # chunk -> engine: DVE 0,1; Pool 2,3,4; PE 5,6,7
        def emit_pe_mix(ch):
            base = ch * P
            ps_m = psp.tile([P, D], F32, tag="s", bufs=4, name=f"psm{ch}")
            for k in range(KTOP):
                lhs = ident if k == 0 else sid[:, k, :]
                nc.tensor.matmul(
                    ps_m[:],
                    lhs,
                    vt_sb[:, :, bass.ds(dks[k] + base, P)],
                    start=(k == 0),
                    stop=(k == KTOP - 1),
                )
            nc.scalar.copy(mixs[:, ch, :, :], ps_m[:])

        def emit_stt_mix(ch, e, k0_act):
            base = ch * P
            mslice = mixs[:, ch, :, :]
            if k0_act:
                nc.scalar.copy(mslice, vt_sb[:, :, bass.ds(dks[0] + base, P)])
            else:
                e.tensor_copy(mslice, vt_sb[:, :, bass.ds(dks[0] + base, P)])
            for k in range(1, KTOP):
                e.scalar_tensor_tensor(
                    mslice,
                    vt_sb[:, :, bass.ds(dks[k] + base, P)],
                    wbc[:, k : k + 1],
                    mslice,
                    ALU.mult,
                    ALU.add,
                )

        def emit_final(ch, slot):
            ps_o = psp.tile([P, D], F32, tag="s", bufs=4, name=f"pso{ch}")
            for ft in range(FT):
                nc.tensor.matmul(
                    ps_o[:],
                    mixs[:, ch, ft, :],
                    w2[:, ft, :],
                    start=(ft == 0),
                    stop=(ft == FT - 1),
                )
            stg = ostg[:, slot % 3, :]
            nc.scalar.mul(stg, ps_o[:], rse_bc[:])
            nc.sync.dma_start(out_d[ch * P : (ch + 1) * P, :], stg)

        emit_pe_mix(5)
        emit_stt_mix(2, nc.gpsimd, k0_act=True)
        emit_stt_mix(0, nc.vector, k0_act=False)
        emit_pe_mix(6)
        emit_stt_mix(3, nc.gpsimd, k0_act=True)
        emit_pe_mix(7)
        emit_stt_mix(4, nc.gpsimd, k0_act=True)
        emit_stt_mix(1, nc.vector, k0_act=False)

        # finals in expected mix-readiness order
        for slot, ch in enumerate([5, 6, 2, 7, 0, 3, 4, 1]):
            emit_final(ch, slot)

    return nc


_NC = None
TRACE = False
_LAST_RESULTS = None


def _get_nc():
    global _NC
    if _NC is None:
        _NC = _build()
        _NC.finalize()
    return _NC


def _consts():
    import ml_dtypes

    cbf = np.zeros((P, 257), ml_dtypes.bfloat16)
    cbf[:, 0:128] = np.eye(P, dtype=np.float32)
    cbf[:, 128:257] = 1.0
    cf32 = np.ones((1, P), np.float32)
    return cbf, cf32


def kernel(queries, keys, values, wq, wk, wv, wo):
    import ml_dtypes

    nc = _get_nc()
    bf = ml_dtypes.bfloat16
    wq_b = np.ascontiguousarray(wq, dtype=bf)
    wkT_b = np.ascontiguousarray(wk.T, dtype=bf)
    wvT_b = np.ascontiguousarray(wv.T, dtype=bf)
    wo_b = np.ascontiguousarray(wo, dtype=bf)
    cbf, cf32 = _consts()
    in_maps = []
    for c in range(8):
        b, h = divmod(c, 2)
        vrot = np.roll(values[b], -h * HALF, axis=0)
        in_maps.append(
            {
                "q": np.ascontiguousarray(queries[b], dtype=bf),
                "kt": np.ascontiguousarray(keys[b].T, dtype=bf),
                "vt": np.ascontiguousarray(vrot.T, dtype=bf),
                "wq": wq_b,
                "wkT": wkT_b,
                "wvT": wvT_b,
                "wo": wo_b,
                "cbf": cbf,
                "cf32": cf32,
            }
        )
    global _LAST_RESULTS
    res = run_bass_kernel_spmd(nc, in_maps, list(range(8)), trace=TRACE)
    _LAST_RESULTS = res
    out = np.empty((B, L, D), np.float32)
    for c in range(8):
        b, h = divmod(c, 2)
        out[b, h * HALF : (h + 1) * HALF] = res.results[c]["out"]
    return out
